# revision 45
# baseline (speedup 1.0000x reference)
"""Trainium2 Bass kernel for nn_AttentionProjector (8-core SPMD), v7.

Math: out = softmax(q @ (x@Wk.T).T) @ (x@Wv.T) + Wv_b
Rewritten (Wk_b cancels in softmax):
    q'     = q @ Wk                    [L, D]
    scores = q' @ x.T                  [L, N]
    out    = (softmax(scores) @ x) @ Wv.T + Wv_b
~52 GFLOP total, 6.45 GFLOP/core. All matmuls fp16 (f32 accumulate):
full PE rate, half the HBM traffic of f32, 10 mantissa bits is enough
for the near-one-hot softmax (host-validated rel err ~5e-3 vs 2e-2).

Sharding (8 cores) - all collectives are AllGathers, and the softmax
front is software-pipelined over the two 128-row l-tiles so the small
collectives hide behind the other tile's matmuls:
  phase 1: q'T slice [512, L] per core (Wk cols sharded)
           -> AG-q' split per l-tile; phase 2 l-tile 0 starts as soon
           as its half has gathered, l-tile 1's AG rides behind it.
  phase 2: scores[l, n_j] (token dim sharded), l-tile-major;
           after each l-tile: local max -> tiny AG-m for that tile
           (overlapped with the exp, which uses the LOCAL max, and
           with the other l-tile's matmuls).
  rescale: each core scales its own p by exp(m_loc - M), folded into
           the p-transpose matmul via a diagonal matrix. Local sums
           ride inside the AG-pT payload (f32 bitcast into trailing
           columns), so S = sum_j s_j exp(m_j - M) needs no extra
           collective. AG-pT buffers are uint16: a float-typed
           transport flushes fp16-denormal-looking bit patterns.
  phase 3: uT[ds_j, L] full contraction over n using a column slice
           x[:, ds_j] and the AG'd pT -> no AllReduce -> AG-uT
  phase 4: out[:, ds_j] = (uT/S).T @ Wv[ds_j,:].T + Wv_b[ds_j]

All HBM inputs are host-pre-tiled to [128, F] with each partition's
bytes contiguous (8-16KB DMA descriptors). qts/wk/xt/wvt are fully
resident, loaded unconditionally on the sync ring in phase order;
xc streams 2-buffered behind them. Bounce writes ride the scalar
ring; collectives + readbacks the gpsimd (SWDGE) path.
"""

import numpy as np

L = 256          # query rows
D = 4096         # d_in == d_out
N = 8192         # tokens
NCORES = 8
NS = N // NCORES     # 1024 tokens per core
DS = D // NCORES     # 512 d-slice per core

LT = L // 128        # 2 l-tiles
DT = D // 128        # 32 d-tiles
NTL = NS // 128      # 8 local n-tiles
NTA = N // 128       # 64 global n-tiles

_MAX_WAITS = 1


def _split_waits(nc, mybir, bass_rust):
    """Walrus in this container allows only one sync-wait per instruction;
    move excess waits onto preceding same-engine no-ops."""
    for bb in nc.main_func.blocks:
        new_list = []
        for ins in bb.instructions:
            si = ins.sync_info
            waits = list(si.on_wait) if si is not None else []
            if len(waits) > _MAX_WAITS:
                for i in range(_MAX_WAITS, len(waits), _MAX_WAITS):
                    nop = mybir.InstNoOp(name=f"{ins.name}-wsplit{i}", ins=[], outs=[])
                    nop.engine = ins.engine
                    nop.sync_info = bass_rust.SyncInfo(
                        on_wait=waits[i:i + _MAX_WAITS], on_update=[])
                    new_list.append(nop)
                ins.sync_info = bass_rust.SyncInfo(
                    on_wait=waits[:_MAX_WAITS], on_update=si.on_update)
            new_list.append(ins)
        bb.instructions[:] = new_list


_NC = None


def _build(split_waits=True):
    global _NC
    if _NC is not None and split_waits:
        return _NC
    import bass_rust
    import concourse.bass as bass
    import concourse.mybir as mybir
    import concourse.tile as tile
    from concourse.masks import make_identity
    from contextlib import ExitStack

    f32 = mybir.dt.float32
    f16 = mybir.dt.float16
    u16 = mybir.dt.uint16
    AF = mybir.ActivationFunctionType
    AX = mybir.AxisListType
    ALU = mybir.AluOpType
    RG = [list(range(NCORES))]

    nc = bass.Bass()

    PF = NTL * L + 64    # pT payload + ms ride-along tail (64B-aligned rows)

    # per-core external I/O (host pre-tiled, see kernel() below)
    t_qts = nc.dram_tensor("qts", [128, DT * L], f16, kind="ExternalInput")
    t_wk = nc.dram_tensor("wk", [128, DT * DS], f16, kind="ExternalInput")
    t_xt = nc.dram_tensor("xt", [128, DT * NS], f16, kind="ExternalInput")
    t_xc = nc.dram_tensor("xc", [128, NTA * DS], f16, kind="ExternalInput")
    t_wvt = nc.dram_tensor("wvt", [128, DT * DS], f16, kind="ExternalInput")
    t_wvb = nc.dram_tensor("wvb", [1, DS], f32, kind="ExternalInput")
    t_out = nc.dram_tensor("out", [L, DS], f32, kind="ExternalOutput")

    # collective bounce buffers (input Local, output Shared)
    agq_i = [nc.dram_tensor(f"agq_i{i}", [128, 4 * 128], f16) for i in range(LT)]
    agq_o = [nc.dram_tensor(f"agq_o{i}", [128 * NCORES, 4 * 128], f16,
                            addr_space="Shared") for i in range(LT)]
    agm_i = [nc.dram_tensor(f"agm_i{i}", [1, 128], f32) for i in range(LT)]
    agm_o = [nc.dram_tensor(f"agm_o{i}", [NCORES, 128], f32,
                            addr_space="Shared") for i in range(LT)]
    PFA = 4 * L + 64     # half-pT payload + ms ride-along tail
    agp_ia = nc.dram_tensor("agp_ia", [128, 4 * L], u16)
    agp_oa = nc.dram_tensor("agp_oa", [128 * NCORES, 4 * L], u16, addr_space="Shared")
    agp_ib = nc.dram_tensor("agp_ib", [128, PFA], u16)
    agp_ob = nc.dram_tensor("agp_ob", [128 * NCORES, PFA], u16, addr_space="Shared")
    agu_ia = nc.dram_tensor("agu_ia", [128, 2 * L], f16)
    agu_oa = nc.dram_tensor("agu_oa", [128 * NCORES, 2 * L], f16, addr_space="Shared")
    agu_ib = nc.dram_tensor("agu_ib", [128, 2 * L], f16)
    agu_ob = nc.dram_tensor("agu_ob", [128 * NCORES, 2 * L], f16, addr_space="Shared")

    qts_re = t_qts.ap().rearrange("p (t l) -> p t l", t=DT)     # [128, 32, 256]
    wk_re = t_wk.ap().rearrange("p (t d) -> p t d", t=DT)       # [128, 32, 512]
    xt_re = t_xt.ap().rearrange("p (t n) -> p t n", t=DT)       # [128, 32, 1024]
    xc_re = t_xc.ap().rearrange("p (t d) -> p t d", t=NTA)      # [128, 64, 512]
    wvt_re = t_wvt.ap().rearrange("p (t o) -> p t o", t=DT)     # [128, 32, 512]
    agqo_re = [t.ap().rearrange("(r p) (t l) -> p r t l", p=128, t=4)
               for t in agq_o]
    agpo_rea = agp_oa.ap().rearrange("(r p) (t l) -> p r t l", p=128, t=NTL)
    agpo_reb = agp_ob.ap().rearrange("(r p) f -> p r f", p=128)
    aguo_rea = agu_oa.ap().rearrange("(r p) (t l) -> p r t l", p=128, t=2)
    aguo_reb = agu_ob.ap().rearrange("(r p) (t l) -> p r t l", p=128, t=2)

    with ExitStack() as ctx:
        tc = ctx.enter_context(tile.TileContext(nc))
        const = ctx.enter_context(tc.tile_pool(name="const", bufs=1))
        small = ctx.enter_context(tc.tile_pool(name="small", bufs=1))

        # Pools opened in reverse-lifetime (stack) order: wv lives to ph4,
        # xt to ph2 end, qpT to ph2 end, ph1 (qts+wk) to ph1 end. DMA ring
        # order (= textual dma_start order) stays qts, wk, xt, wv.
        wv_cm = tc.tile_pool(name="wv", bufs=1)
        wvp = wv_cm.__enter__()
        wv_sb = wvp.tile([128, DT, DS], f16)
        xt_cm = tc.tile_pool(name="xt", bufs=1)
        xtp = xt_cm.__enter__()
        xt_sb = xtp.tile([128, DT, NS], f16)
        qpT_cm = tc.tile_pool(name="qpTp", bufs=1)
        qpTp = qpT_cm.__enter__()
        qpT = qpTp.tile([128, LT, DT, 128], f16, name="qpT")     # full q'T (2MB)
        ph1_cm = tc.tile_pool(name="ph1", bufs=1)
        ph1 = ph1_cm.__enter__()
        qts_sb = ph1.tile([128, DT, L], f16)
        wk_sb = ph1.tile([128, DT, DS], f16)

        for c in range(2):
            nc.sync.dma_start(qts_sb[:, c * 16:(c + 1) * 16, :],
                              qts_re[:, c * 16:(c + 1) * 16, :])
            for k in range(2):
                w = c * 2 + k
                nc.sync.dma_start(wk_sb[:, w * 8:(w + 1) * 8, :],
                                  wk_re[:, w * 8:(w + 1) * 8, :])
        for c in range(4):
            nc.sync.dma_start(xt_sb[:, c * 8:(c + 1) * 8, :],
                              xt_re[:, c * 8:(c + 1) * 8, :])
        for c in range(4):
            nc.sync.dma_start(wv_sb[:, c * 8:(c + 1) * 8, :],
                              wvt_re[:, c * 8:(c + 1) * 8, :])


        # constants
        ident16 = const.tile([128, 128], f16)
        make_identity(nc, ident16[:])
        ident32 = const.tile([128, 128], f32)
        make_identity(nc, ident32[:])
        bias_sb = const.tile([128, DS], f32)
        nc.scalar.dma_start(bias_sb[:],
                            t_wvb.ap().partition_broadcast(128)[:, 0, :])

        p_sb = small.tile([128, LT, NS], f16, name="p_sb")       # local p (0.5MB)
        pT_loc = small.tile([128, PF], f16, name="pT_loc")       # pT + ms tail
        uT_loc = small.tile([128, 4, L], f16, name="uT_loc")

        # ------------- phase 1: q'T slice = Wk[:, ds_j].T @ q.T --------------
        with tc.tile_pool(name="ph1ps", bufs=1, space="PSUM") as ph1ps:
            ps1 = [ph1ps.tile([128, L], f32, name=f"ps1_{i}") for i in range(4)]
            qpT_l = ph1.tile([128, LT, 4, 128], f16)
            for kt in range(DT):
                for dtl in range(4):
                    nc.tensor.matmul(
                        ps1[dtl][:], wk_sb[:, kt, dtl * 128:(dtl + 1) * 128],
                        qts_sb[:, kt, :], start=(kt == 0), stop=(kt == DT - 1))
            for dtl in range(4):
                for lt in range(LT):
                    nc.vector.tensor_copy(qpT_l[:, lt, dtl, :],
                                          ps1[dtl][:, lt * 128:(lt + 1) * 128])
        for lt in range(LT):
            nc.scalar.dma_start(
                agq_i[lt].ap().rearrange("p (t l) -> p t l", t=4),
                qpT_l[:, lt, :, :])
        for lt in range(LT):
            nc.gpsimd.collective_compute(
                "AllGather", ALU.bypass, replica_groups=RG,
                ins=[agq_i[lt].ap().opt()], outs=[agq_o[lt].ap().opt()])
            for c in range(2):
                nc.gpsimd.dma_start(qpT[:, lt, :, :]
                                    .rearrange("p (r t) l -> p r t l", r=NCORES)
                                    [:, c * 4:(c + 1) * 4, :, :],
                                    agqo_re[lt][:, c * 4:(c + 1) * 4, :, :])
        ph1_cm.__exit__(None, None, None)

        # ------------- phase 2: scores[l, n_j], l-tile-major -----------------
        m_loc = small.tile([128, LT], f32, name="m_loc")
        s_loc = small.tile([128, LT], f32, name="s_loc")
        neg_m = small.tile([128, LT], f32, name="neg_m")
        Mg = small.tile([128, LT], f32, name="Mg")
        negMg = small.tile([128, LT], f32, name="negMg")
        f_me = small.tile([128, LT], f32, name="f_me")
        diag = small.tile([128, LT, 128], f16, name="diag")
        m_all = small.tile([128, LT, NCORES], f32, name="m_all")

        ph2ps_cm = tc.tile_pool(name="ph2ps", bufs=1, space="PSUM")
        ph2ps = ph2ps_cm.__enter__()
        scps_cm = tc.tile_pool(name="scps", bufs=2, space="PSUM")
        scps = scps_cm.__enter__()
        score_ps = [[ph2ps.tile([128, 512], f32, name=f"sc{i}_{k}")
                     for k in range(2)] for i in range(LT)]

        def softmax_front(lt):
            """local max -> AG-m(lt) -> exp with local max (overlapped)."""
            mtmp = small.tile([128, 1], f32, name=f"mtmp{lt}")
            nc.vector.tensor_reduce(mtmp[:], score_ps[lt][0][:],
                                    axis=AX.X, op=ALU.max)
            nc.vector.tensor_reduce(m_loc[:, lt:lt + 1], score_ps[lt][1][:],
                                    axis=AX.X, op=ALU.max)
            nc.vector.tensor_tensor(m_loc[:, lt:lt + 1], m_loc[:, lt:lt + 1],
                                    mtmp[:], ALU.max)
            m_tp = scps.tile([128, 128], f32, name="sc_tp")
            nc.tensor.transpose(m_tp[0:1, :], m_loc[:, lt:lt + 1], ident32[:])
            m_tps = small.tile([1, 128], f32, name=f"m_tps{lt}")
            nc.vector.tensor_copy(m_tps[:], m_tp[0:1, :])
            nc.scalar.dma_start(agm_i[lt].ap(), m_tps[:])
            nc.gpsimd.collective_compute(
                "AllGather", ALU.bypass, replica_groups=RG,
                ins=[agm_i[lt].ap().opt()], outs=[agm_o[lt].ap().opt()])
            # exp with LOCAL max while AG-m is in flight
            nc.vector.tensor_scalar_mul(neg_m[:, lt:lt + 1],
                                        m_loc[:, lt:lt + 1], -1.0)
            sp0 = small.tile([128, 1], f32, name=f"sp0_{lt}")
            nc.scalar.activation(p_sb[:, lt, 0:512], score_ps[lt][0][:],
                                 AF.Exp, bias=neg_m[:, lt:lt + 1],
                                 accum_out=sp0[:])
            nc.scalar.activation(p_sb[:, lt, 512:1024], score_ps[lt][1][:],
                                 AF.Exp, bias=neg_m[:, lt:lt + 1],
                                 accum_out=s_loc[:, lt:lt + 1])
            nc.vector.tensor_tensor(s_loc[:, lt:lt + 1], s_loc[:, lt:lt + 1],
                                    sp0[:], ALU.add)

        def softmax_back(lt):
            """read back gathered m; global M, rescale factor, diag."""
            m_all_t = small.tile([NCORES, 128], f32, name=f"m_all_t{lt}")
            nc.gpsimd.dma_start(m_all_t[:], agm_o[lt].ap())
            m_tp2 = scps.tile([128, 128], f32, name="sc_tp")
            nc.tensor.transpose(m_tp2[:, 0:NCORES], m_all_t[:],
                                ident32[0:NCORES, 0:NCORES])
            nc.vector.tensor_copy(m_all[:, lt, :], m_tp2[:, 0:NCORES])
            nc.vector.tensor_reduce(Mg[:, lt:lt + 1], m_all[:, lt, :],
                                    axis=AX.X, op=ALU.max)
            nc.vector.tensor_scalar_mul(negMg[:, lt:lt + 1], Mg[:, lt:lt + 1],
                                        -1.0)
            nc.scalar.activation(f_me[:, lt:lt + 1], m_loc[:, lt:lt + 1],
                                 AF.Exp, bias=negMg[:, lt:lt + 1])
            nc.scalar.activation(diag[:, lt, :], ident16[:], AF.Copy,
                                 scale=f_me[:, lt:lt + 1])

        pT_part = pT_loc[:][:, 0:NTL * L].rearrange(
            "p (lt t l) -> p lt t l", lt=LT, t=NTL)

        def transpose_p(lt, n0=0, n1=NTL):
            """transpose-and-rescale p via matmul against diag(f_me)."""
            for ntl in range(n0, n1):
                tp = scps.tile([128, 128], f32, name="sc_tp")
                nc.tensor.matmul(
                    tp[:], p_sb[:, lt, ntl * 128:(ntl + 1) * 128],
                    diag[:, lt, :], start=True, stop=True)
                nc.vector.tensor_copy(pT_part[:, lt, ntl, :], tp[:])

        # lt0 matmuls
        for dt in range(DT):
            for nh in range(2):
                nc.tensor.matmul(
                    score_ps[0][nh][:], qpT[:, 0, dt, :],
                    xt_sb[:, dt, nh * 512:(nh + 1) * 512],
                    start=(dt == 0), stop=(dt == DT - 1))
        softmax_front(0)     # AG-m(0) + exp(0) overlap lt1 matmuls below
        # lt1 matmuls
        for dt in range(DT):
            for nh in range(2):
                nc.tensor.matmul(
                    score_ps[1][nh][:], qpT[:, 1, dt, :],
                    xt_sb[:, dt, nh * 512:(nh + 1) * 512],
                    start=(dt == 0), stop=(dt == DT - 1))
        softmax_back(0)      # rb-m0 queued before AG-m(1)'s blocking wait
        transpose_p(0)
        # lt0's pT half leaves immediately: its mesh overlaps the whole
        # lt1 softmax chain (AG-m1, exp, diag, transposes) below
        nc.scalar.dma_start(agp_ia.ap(),
                            pT_loc[:][:, 0:4 * L].bitcast(u16))
        nc.gpsimd.collective_compute(
            "AllGather", ALU.bypass, replica_groups=RG,
            ins=[agp_ia.ap().opt()], outs=[agp_oa.ap().opt()])
        softmax_front(1)
        softmax_back(1)
        transpose_p(1)
        # lt1 half + (m, s) stats ride-along
        s_pack = small.tile([128, 2 * LT], f32, name="s_pack")
        nc.vector.tensor_copy(s_pack[:, 0:LT], m_loc[:])
        nc.vector.tensor_copy(s_pack[:, LT:2 * LT], s_loc[:])
        nc.vector.tensor_copy(pT_loc[:][:, NTL * L:NTL * L + 8].bitcast(u16),
                              s_pack[:].bitcast(u16))
        nc.scalar.dma_start(agp_ib.ap()[:, 0:4 * L],
                            pT_loc[:][:, 4 * L:NTL * L].bitcast(u16))
        nc.scalar.dma_start(agp_ib.ap()[:, 4 * L:4 * L + 8],
                            pT_loc[:][:, NTL * L:NTL * L + 8].bitcast(u16))
        nc.gpsimd.collective_compute(
            "AllGather", ALU.bypass, replica_groups=RG,
            ins=[agp_ib.ap().opt()], outs=[agp_ob.ap().opt()])
        scps_cm.__exit__(None, None, None)
        ph2ps_cm.__exit__(None, None, None)

        qpT_cm.__exit__(None, None, None)
        xt_cm.__exit__(None, None, None)

        # ------------- phase 3: uT[ds_j, L] = x[:, ds_j].T @ pT (full n) -----
        # pT is idx-ordered: idx = h*32 + r*4 + t  <->  global nt = 8r + 4h + t
        # (host permutes xc to match, so consumption is sequential in idx).
        pTp_cm = tc.tile_pool(name="pTp", bufs=1)
        pTp = pTp_cm.__enter__()
        pT = pTp.tile([128, LT, NTA, 128], f16, name="pT")       # full pT (4MB)
        pT5 = pT[:].bitcast(u16).rearrange("p lt (r t) l -> p lt r t l", r=8)
        agpo_pb = agpo_reb[:, :, 0:4 * L].rearrange(
            "p r (t l) -> p r t l", t=NTL)
        for c in range(4):
            nc.gpsimd.dma_start(pT5[:, 0, c * 2:(c + 1) * 2, :, :],
                                agpo_rea[:, c * 2:(c + 1) * 2, :, :])
        for c in range(4):
            nc.gpsimd.dma_start(pT5[:, 1, c * 2:(c + 1) * 2, :, :],
                                agpo_pb[:, c * 2:(c + 1) * 2, :, :])
        ms32 = small.tile([128, 32], f32, name="ms32")
        nc.gpsimd.dma_start(
            ms32[:].bitcast(u16).rearrange("p (r c) -> p r c", r=8),
            agpo_reb[:, :, 4 * L:4 * L + 8])

        with tc.tile_pool(name="ph3xc", bufs=1) as ph3xc, \
             tc.tile_pool(name="ph3ps", bufs=1, space="PSUM") as ph3ps:
            xc_sb = ph3xc.tile([128, NTA, DS], f16)
            for c in range(4):
                nc.sync.dma_start(xc_sb[:, c * 16:(c + 1) * 16, :],
                                  xc_re[:, c * 16:(c + 1) * 16, :])
            psu = [ph3ps.tile([128, LT, 128], f32, name=f"psu{i}")
                   for i in range(4)]
            # pass 1: d-column tiles 0-1 over all n -> uT half a early, so
            # AG-uT-a's mesh overlaps pass 2 below
            for idx in range(NTA):
                for ci in range(2):
                    nc.tensor.matmul(
                        psu[ci][:], xc_sb[:, idx, ci * 128:(ci + 1) * 128],
                        pT[:, :, idx, :], start=(idx == 0),
                        stop=(idx == NTA - 1))
            for ci in range(2):
                nc.vector.tensor_copy(
                    uT_loc[:, ci, :].rearrange("p (a b) -> p a b", a=LT),
                    psu[ci][:])
            nc.scalar.dma_start(agu_ia.ap().rearrange("p (t l) -> p t l", t=2),
                                uT_loc[:, 0:2, :])
            nc.gpsimd.collective_compute(
                "AllGather", ALU.bypass, replica_groups=RG,
                ins=[agu_ia.ap().opt()], outs=[agu_oa.ap().opt()])
            # pass 2: d-column tiles 2-3 (re-streams resident xc/pT)
            for idx in range(NTA):
                for ci in range(2, 4):
                    nc.tensor.matmul(
                        psu[ci][:], xc_sb[:, idx, ci * 128:(ci + 1) * 128],
                        pT[:, :, idx, :], start=(idx == 0),
                        stop=(idx == NTA - 1))
            for ci in range(2, 4):
                nc.vector.tensor_copy(
                    uT_loc[:, ci, :].rearrange("p (a b) -> p a b", a=LT),
                    psu[ci][:])
            nc.scalar.dma_start(agu_ib.ap().rearrange("p (t l) -> p t l", t=2),
                                uT_loc[:, 2:4, :])
            nc.gpsimd.collective_compute(
                "AllGather", ALU.bypass, replica_groups=RG,
                ins=[agu_ib.ap().opt()], outs=[agu_ob.ap().opt()])
        pTp_cm.__exit__(None, None, None)

        # S = sum_j s_j * exp(m_j - M) from the AG-pT ride-along; 1/S
        ms_all = ms32[:].rearrange("p (r c) -> p r c", r=8)
        f_all = small.tile([128, 8, LT], f32, name="f_all")
        Sg = small.tile([128, LT], f32, name="Sg")
        rec = small.tile([128, LT], f32, name="rec")
        for lt in range(LT):
            nc.scalar.activation(f_all[:, :, lt:lt + 1], ms_all[:, :, lt:lt + 1],
                                 AF.Exp, bias=negMg[:, lt:lt + 1])
            nc.vector.tensor_tensor(f_all[:, :, lt:lt + 1],
                                    f_all[:, :, lt:lt + 1],
                                    ms_all[:, :, LT + lt:LT + lt + 1], ALU.mult)
            nc.vector.tensor_reduce(Sg[:, lt:lt + 1], f_all[:, :, lt:lt + 1],
                                    axis=AX.XY, op=ALU.add)
        nc.vector.reciprocal(rec[:], Sg[:])

        # ------------- phase 4: out = (ctxT/S).T @ WvT + Wv_b ----------------
        ctx_cm = tc.tile_pool(name="ctxp", bufs=1)
        ctxp = ctx_cm.__enter__()
        # ctxT is idx-ordered: idx = h*16 + r*2 + t  <->  global dt = 4r+2h+t
        ctxT = ctxp.tile([128, DT, L], f16, name="ctxT")         # full uT (2MB)
        ctx5 = ctxT[:].rearrange("p (h r t) l -> p h r t l", h=2, r=8)
        for c in range(4):
            nc.gpsimd.dma_start(ctx5[:, 0, c * 2:(c + 1) * 2, :, :],
                                aguo_rea[:, c * 2:(c + 1) * 2, :, :])
        for c in range(4):
            nc.gpsimd.dma_start(ctx5[:, 1, c * 2:(c + 1) * 2, :, :],
                                aguo_reb[:, c * 2:(c + 1) * 2, :, :])
        with tc.tile_pool(name="ph4ps", bufs=1, space="PSUM") as ph4ps, \
             tc.tile_pool(name="ph4o", bufs=2) as ph4o:
            po = [ph4ps.tile([128, DS], f32, name=f"po{i}") for i in range(LT)]
            for h in range(2):
                for r in range(NCORES):
                    for t in range(2):
                        idx = h * 16 + r * 2 + t
                        dt = 4 * r + 2 * h + t
                        first = (h == 0 and r == 0 and t == 0)
                        last = (h == 1 and r == NCORES - 1 and t == 1)
                        for lt in range(LT):
                            nc.tensor.matmul(
                                po[lt][:],
                                ctxT[:, idx, lt * 128:(lt + 1) * 128],
                                wv_sb[:, dt, :], start=first, stop=last)
            for lt in range(LT):
                o_sb = ph4o.tile([128, DS], f32)
                nc.scalar.activation(o_sb[:], po[lt][:], AF.Copy,
                                     scale=rec[:, lt:lt + 1])
                nc.vector.tensor_tensor(o_sb[:], o_sb[:], bias_sb[:], ALU.add)
                nc.scalar.dma_start(t_out[lt * 128:(lt + 1) * 128, :], o_sb[:])
        ctx_cm.__exit__(None, None, None)
        wv_cm.__exit__(None, None, None)

    if split_waits:
        _split_waits(nc, mybir, bass_rust)
        _NC = nc
    return nc


def _tile128(a):
    """[T*128, F] -> [128, T*F] so each partition's bytes are contiguous."""
    t = a.shape[0] // 128
    return np.ascontiguousarray(
        a.reshape(t, 128, a.shape[1]).transpose(1, 0, 2).reshape(128, -1))


# phase-3 consumption order: idx = h*32 + r*4 + t <-> global nt = 8r + 4h + t
XC_PERM = [8 * r + 4 * h + t for h in range(2) for r in range(8)
           for t in range(4)]

last_results = None


def kernel(src_prompts, query, Wk_w, Wk_b, Wv_w, Wv_b):
    global last_results
    from concourse.bass_utils import run_bass_kernel_spmd

    nc = _build()

    x = np.asarray(src_prompts, dtype=np.float32)[0]
    q = np.asarray(query, dtype=np.float32)
    wk = np.asarray(Wk_w, dtype=np.float32)
    wv = np.asarray(Wv_w, dtype=np.float32)
    wvb = np.asarray(Wv_b, dtype=np.float32)
    # Wk_b shifts every score row by a constant -> cancels in softmax.

    x16 = x.astype(np.float16)
    qts = _tile128(np.ascontiguousarray(q.T).astype(np.float16))
    in_maps = []
    for j in range(NCORES):
        ns, ds = slice(j * NS, (j + 1) * NS), slice(j * DS, (j + 1) * DS)
        in_maps.append({
            "qts": qts,
            "wk": _tile128(wk[:, ds].astype(np.float16)),
            "xt": _tile128(np.ascontiguousarray(x16[ns].T)),
            "xc": _tile128(np.ascontiguousarray(x16[:, ds])),
            "wvt": _tile128(np.ascontiguousarray(wv[ds].T).astype(np.float16)),
            "wvb": np.ascontiguousarray(wvb[ds][None, :]),
        })

    res = run_bass_kernel_spmd(nc, in_maps, core_ids=list(range(NCORES)))
    last_results = res
    out = np.concatenate([res.results[j]["out"] for j in range(NCORES)], axis=1)
    return out[None, :, :]


# revision 47
# speedup vs baseline: 1.0661x; 1.0661x over previous
"""Trainium2 Bass kernel for nn_AttentionProjector (8-core SPMD), v7.

Math: out = softmax(q @ (x@Wk.T).T) @ (x@Wv.T) + Wv_b
Rewritten (Wk_b cancels in softmax):
    q'     = q @ Wk                    [L, D]
    scores = q' @ x.T                  [L, N]
    out    = (softmax(scores) @ x) @ Wv.T + Wv_b
~52 GFLOP total, 6.45 GFLOP/core. All matmuls fp16 (f32 accumulate):
full PE rate, half the HBM traffic of f32, 10 mantissa bits is enough
for the near-one-hot softmax (host-validated rel err ~5e-3 vs 2e-2).

Sharding (8 cores) - all collectives are AllGathers, and the softmax
front is software-pipelined over the two 128-row l-tiles so the small
collectives hide behind the other tile's matmuls:
  phase 1: q'T slice [512, L] per core (Wk cols sharded)
           -> AG-q' split per l-tile; phase 2 l-tile 0 starts as soon
           as its half has gathered, l-tile 1's AG rides behind it.
  phase 2: scores[l, n_j] (token dim sharded), l-tile-major;
           after each l-tile: local max -> tiny AG-m for that tile
           (overlapped with the exp, which uses the LOCAL max, and
           with the other l-tile's matmuls).
  rescale: each core scales its own p by exp(m_loc - M), folded into
           the p-transpose matmul via a diagonal matrix. Local sums
           ride inside the AG-pT payload (f32 bitcast into trailing
           columns), so S = sum_j s_j exp(m_j - M) needs no extra
           collective. AG-pT buffers are uint16: a float-typed
           transport flushes fp16-denormal-looking bit patterns.
  phase 3: uT[ds_j, L] full contraction over n using a column slice
           x[:, ds_j] and the AG'd pT -> no AllReduce -> AG-uT
  phase 4: out[:, ds_j] = (uT/S).T @ Wv[ds_j,:].T + Wv_b[ds_j]

All HBM inputs are host-pre-tiled to [128, F] with each partition's
bytes contiguous (8-16KB DMA descriptors). qts/wk/xt/wvt are fully
resident, loaded unconditionally on the sync ring in phase order;
xc streams 2-buffered behind them. Bounce writes ride the scalar
ring; collectives + readbacks the gpsimd (SWDGE) path.
"""

import numpy as np

L = 256          # query rows
D = 4096         # d_in == d_out
N = 8192         # tokens
NCORES = 8
NS = N // NCORES     # 1024 tokens per core
DS = D // NCORES     # 512 d-slice per core

LT = L // 128        # 2 l-tiles
DT = D // 128        # 32 d-tiles
NTL = NS // 128      # 8 local n-tiles
NTA = N // 128       # 64 global n-tiles

_MAX_WAITS = 1


def _split_waits(nc, mybir, bass_rust):
    """Walrus in this container allows only one sync-wait per instruction;
    move excess waits onto preceding same-engine no-ops."""
    for bb in nc.main_func.blocks:
        new_list = []
        for ins in bb.instructions:
            si = ins.sync_info
            waits = list(si.on_wait) if si is not None else []
            if len(waits) > _MAX_WAITS:
                for i in range(_MAX_WAITS, len(waits), _MAX_WAITS):
                    nop = mybir.InstNoOp(name=f"{ins.name}-wsplit{i}", ins=[], outs=[])
                    nop.engine = ins.engine
                    nop.sync_info = bass_rust.SyncInfo(
                        on_wait=waits[i:i + _MAX_WAITS], on_update=[])
                    new_list.append(nop)
                ins.sync_info = bass_rust.SyncInfo(
                    on_wait=waits[:_MAX_WAITS], on_update=si.on_update)
            new_list.append(ins)
        bb.instructions[:] = new_list


_NC = None


def _build(split_waits=True):
    global _NC
    if _NC is not None and split_waits:
        return _NC
    import bass_rust
    import concourse.bass as bass
    import concourse.mybir as mybir
    import concourse.tile as tile
    from concourse.masks import make_identity
    from contextlib import ExitStack

    f32 = mybir.dt.float32
    f16 = mybir.dt.float16
    u16 = mybir.dt.uint16
    AF = mybir.ActivationFunctionType
    AX = mybir.AxisListType
    ALU = mybir.AluOpType
    RG = [list(range(NCORES))]

    nc = bass.Bass()

    PF = NTL * L + 64    # pT payload + ms ride-along tail (64B-aligned rows)

    # per-core external I/O (host pre-tiled, see kernel() below)
    t_qts = nc.dram_tensor("qts", [128, DT * L], f16, kind="ExternalInput")
    t_wk = nc.dram_tensor("wk", [128, DT * DS], f16, kind="ExternalInput")
    t_xt = nc.dram_tensor("xt", [128, DT * NS], f16, kind="ExternalInput")
    t_xc = nc.dram_tensor("xc", [128, NTA * DS], f16, kind="ExternalInput")
    t_wvt = nc.dram_tensor("wvt", [128, DT * DS], f16, kind="ExternalInput")
    t_wvb = nc.dram_tensor("wvb", [1, DS], f32, kind="ExternalInput")
    t_out = nc.dram_tensor("out", [L, DS], f32, kind="ExternalOutput")

    # collective bounce buffers (input Local, output Shared)
    agq_i = [nc.dram_tensor(f"agq_i{i}", [128, 4 * 128], f16) for i in range(LT)]
    agq_o = [nc.dram_tensor(f"agq_o{i}", [128 * NCORES, 4 * 128], f16,
                            addr_space="Shared") for i in range(LT)]
    agm_i = nc.dram_tensor("agm_i", [LT, 128], f32)
    agm_o = nc.dram_tensor("agm_o", [LT * NCORES, 128], f32,
                           addr_space="Shared")
    PFA = 4 * L + 64     # half-pT payload + ms ride-along tail
    agp_ia = nc.dram_tensor("agp_ia", [128, PFA], u16)
    agp_oa = nc.dram_tensor("agp_oa", [128 * NCORES, PFA], u16, addr_space="Shared")
    agp_ib = nc.dram_tensor("agp_ib", [128, 4 * L], u16)
    agp_ob = nc.dram_tensor("agp_ob", [128 * NCORES, 4 * L], u16, addr_space="Shared")
    agu_ia = nc.dram_tensor("agu_ia", [128, 2 * L], f16)
    agu_oa = nc.dram_tensor("agu_oa", [128 * NCORES, 2 * L], f16, addr_space="Shared")
    agu_ib = nc.dram_tensor("agu_ib", [128, 2 * L], f16)
    agu_ob = nc.dram_tensor("agu_ob", [128 * NCORES, 2 * L], f16, addr_space="Shared")

    qts_re = t_qts.ap().rearrange("p (t l) -> p t l", t=DT)     # [128, 32, 256]
    wk_re = t_wk.ap().rearrange("p (t d) -> p t d", t=DT)       # [128, 32, 512]
    xt_re = t_xt.ap().rearrange("p (t n) -> p t n", t=DT)       # [128, 32, 1024]
    xc_re = t_xc.ap().rearrange("p (t d) -> p t d", t=NTA)      # [128, 64, 512]
    wvt_re = t_wvt.ap().rearrange("p (t o) -> p t o", t=DT)     # [128, 32, 512]
    agqo_re = [t.ap().rearrange("(r p) (t l) -> p r t l", p=128, t=4)
               for t in agq_o]
    agpo_rea = agp_oa.ap().rearrange("(r p) f -> p r f", p=128)
    agpo_reb = agp_ob.ap().rearrange("(r p) (t l) -> p r t l", p=128, t=4)
    aguo_rea = agu_oa.ap().rearrange("(r p) (t l) -> p r t l", p=128, t=2)
    aguo_reb = agu_ob.ap().rearrange("(r p) (t l) -> p r t l", p=128, t=2)

    with ExitStack() as ctx:
        tc = ctx.enter_context(tile.TileContext(nc))
        const = ctx.enter_context(tc.tile_pool(name="const", bufs=1))
        small = ctx.enter_context(tc.tile_pool(name="small", bufs=1))

        # Pools opened in reverse-lifetime (stack) order: wv lives to ph4,
        # xt to ph2 end, qpT to ph2 end, ph1 (qts+wk) to ph1 end. DMA ring
        # order (= textual dma_start order) stays qts, wk, xt, wv.
        wv_cm = tc.tile_pool(name="wv", bufs=1)
        wvp = wv_cm.__enter__()
        wv_sb = wvp.tile([128, DT, DS], f16)
        xt_cm = tc.tile_pool(name="xt", bufs=1)
        xtp = xt_cm.__enter__()
        xt_sb = xtp.tile([128, DT, NS], f16)
        qpT_cm = tc.tile_pool(name="qpTp", bufs=1)
        qpTp = qpT_cm.__enter__()
        qpT = qpTp.tile([128, LT, DT, 128], f16, name="qpT")     # full q'T (2MB)
        ph1_cm = tc.tile_pool(name="ph1", bufs=1)
        ph1 = ph1_cm.__enter__()
        qts_sb = ph1.tile([128, DT, L], f16)
        wk_sb = ph1.tile([128, DT, DS], f16)

        for c in range(2):
            nc.sync.dma_start(qts_sb[:, c * 16:(c + 1) * 16, :],
                              qts_re[:, c * 16:(c + 1) * 16, :])
            for k in range(2):
                w = c * 2 + k
                nc.sync.dma_start(wk_sb[:, w * 8:(w + 1) * 8, :],
                                  wk_re[:, w * 8:(w + 1) * 8, :])
        for c in range(4):
            nc.sync.dma_start(xt_sb[:, c * 8:(c + 1) * 8, :],
                              xt_re[:, c * 8:(c + 1) * 8, :])
        for c in range(4):
            nc.sync.dma_start(wv_sb[:, c * 8:(c + 1) * 8, :],
                              wvt_re[:, c * 8:(c + 1) * 8, :])


        # constants
        ident16 = const.tile([128, 128], f16)
        make_identity(nc, ident16[:])
        ident32 = const.tile([128, 128], f32)
        make_identity(nc, ident32[:])
        bias_sb = const.tile([128, DS], f32)
        nc.scalar.dma_start(bias_sb[:],
                            t_wvb.ap().partition_broadcast(128)[:, 0, :])

        p_sb = small.tile([128, LT, NS], f16, name="p_sb")       # local p (0.5MB)
        pT_loc = small.tile([128, PF], f16, name="pT_loc")       # pT + ms tail
        uT_loc = small.tile([128, 4, L], f16, name="uT_loc")

        # ------------- phase 1: q'T slice = Wk[:, ds_j].T @ q.T --------------
        with tc.tile_pool(name="ph1ps", bufs=1, space="PSUM") as ph1ps:
            ps1 = [ph1ps.tile([128, L], f32, name=f"ps1_{i}") for i in range(4)]
            qpT_l = ph1.tile([128, LT, 4, 128], f16)
            for kt in range(DT):
                for dtl in range(4):
                    nc.tensor.matmul(
                        ps1[dtl][:], wk_sb[:, kt, dtl * 128:(dtl + 1) * 128],
                        qts_sb[:, kt, :], start=(kt == 0), stop=(kt == DT - 1))
            for dtl in range(4):
                for lt in range(LT):
                    nc.vector.tensor_copy(qpT_l[:, lt, dtl, :],
                                          ps1[dtl][:, lt * 128:(lt + 1) * 128])
        for lt in range(LT):
            nc.scalar.dma_start(
                agq_i[lt].ap().rearrange("p (t l) -> p t l", t=4),
                qpT_l[:, lt, :, :])
        for lt in range(LT):
            nc.gpsimd.collective_compute(
                "AllGather", ALU.bypass, replica_groups=RG,
                ins=[agq_i[lt].ap().opt()], outs=[agq_o[lt].ap().opt()])
            for c in range(2):
                nc.gpsimd.dma_start(qpT[:, lt, :, :]
                                    .rearrange("p (r t) l -> p r t l", r=NCORES)
                                    [:, c * 4:(c + 1) * 4, :, :],
                                    agqo_re[lt][:, c * 4:(c + 1) * 4, :, :])
        ph1_cm.__exit__(None, None, None)

        # ------------- phase 2: scores[l, n_j], l-tile-major -----------------
        m_loc = small.tile([128, LT], f32, name="m_loc")
        s_loc = small.tile([128, LT], f32, name="s_loc")
        neg_m = small.tile([128, LT], f32, name="neg_m")
        Mg = small.tile([128, LT], f32, name="Mg")
        negMg = small.tile([128, LT], f32, name="negMg")
        f_me = small.tile([128, LT], f32, name="f_me")
        diag = small.tile([128, LT, 128], f16, name="diag")

        ph2ps_cm = tc.tile_pool(name="ph2ps", bufs=1, space="PSUM")
        ph2ps = ph2ps_cm.__enter__()
        scps_cm = tc.tile_pool(name="scps", bufs=2, space="PSUM")
        scps = scps_cm.__enter__()
        score_ps = [[ph2ps.tile([128, 512], f32, name=f"sc{i}_{k}")
                     for k in range(2)] for i in range(LT)]

        def softmax_front(lt):
            """local max -> AG-m(lt) -> exp with local max (overlapped)."""
            mtmp = small.tile([128, 1], f32, name=f"mtmp{lt}")
            nc.vector.tensor_reduce(mtmp[:], score_ps[lt][0][:],
                                    axis=AX.X, op=ALU.max)
            nc.vector.tensor_reduce(m_loc[:, lt:lt + 1], score_ps[lt][1][:],
                                    axis=AX.X, op=ALU.max)
            nc.vector.tensor_tensor(m_loc[:, lt:lt + 1], m_loc[:, lt:lt + 1],
                                    mtmp[:], ALU.max)
            # exp with LOCAL max (global max arrives via one merged AG-m)
            nc.vector.tensor_scalar_mul(neg_m[:, lt:lt + 1],
                                        m_loc[:, lt:lt + 1], -1.0)
            sp0 = small.tile([128, 1], f32, name=f"sp0_{lt}")
            nc.scalar.activation(p_sb[:, lt, 0:512], score_ps[lt][0][:],
                                 AF.Exp, bias=neg_m[:, lt:lt + 1],
                                 accum_out=sp0[:])
            nc.scalar.activation(p_sb[:, lt, 512:1024], score_ps[lt][1][:],
                                 AF.Exp, bias=neg_m[:, lt:lt + 1],
                                 accum_out=s_loc[:, lt:lt + 1])
            nc.vector.tensor_tensor(s_loc[:, lt:lt + 1], s_loc[:, lt:lt + 1],
                                    sp0[:], ALU.add)

        def softmax_mexch():
            """one merged AG of both l-tiles' local maxima; M, f_me, diag."""
            m_tp = scps.tile([128, 128], f32, name="sc_tp")
            nc.tensor.transpose(m_tp[0:LT, :], m_loc[:], ident32[:])
            m_tps = small.tile([LT, 128], f32, name="m_tps")
            nc.vector.tensor_copy(m_tps[:], m_tp[0:LT, :])
            nc.scalar.dma_start(agm_i.ap(), m_tps[:])
            nc.gpsimd.collective_compute(
                "AllGather", ALU.bypass, replica_groups=RG,
                ins=[agm_i.ap().opt()], outs=[agm_o.ap().opt()])
            m_all_t = small.tile([LT * NCORES, 128], f32, name="m_all_t")
            nc.gpsimd.dma_start(m_all_t[:], agm_o.ap())
            m_tp2 = scps.tile([128, 128], f32, name="sc_tp")
            nc.tensor.transpose(m_tp2[:, 0:LT * NCORES], m_all_t[:],
                                ident32[0:LT * NCORES, 0:LT * NCORES])
            m_all16 = small.tile([128, LT * NCORES], f32, name="m_all16")
            nc.vector.tensor_copy(m_all16[:], m_tp2[:, 0:LT * NCORES])
            mav = m_all16[:].rearrange("p (r c) -> p r c", r=NCORES)
            for lt in range(LT):
                nc.vector.tensor_reduce(Mg[:, lt:lt + 1], mav[:, :, lt:lt + 1],
                                        axis=AX.XY, op=ALU.max)
            nc.vector.tensor_scalar_mul(negMg[:], Mg[:], -1.0)
            for lt in range(LT):
                nc.scalar.activation(f_me[:, lt:lt + 1], m_loc[:, lt:lt + 1],
                                     AF.Exp, bias=negMg[:, lt:lt + 1])
                nc.scalar.activation(diag[:, lt, :], ident16[:], AF.Copy,
                                     scale=f_me[:, lt:lt + 1])

        pT_part = pT_loc[:][:, 0:NTL * L].rearrange("p (t l) -> p t l", t=NTL)

        def transpose_p(lt, n0=0, n1=NTL):
            """transpose-and-rescale p via matmul against diag(f_me)."""
            for ntl in range(n0, n1):
                tp = scps.tile([128, 128], f32, name="sc_tp")
                nc.tensor.matmul(
                    tp[:], p_sb[:, lt, ntl * 128:(ntl + 1) * 128],
                    diag[:, lt, :], start=True, stop=True)
                nc.vector.tensor_copy(
                    pT_part[:, ntl, lt * 128:(lt + 1) * 128], tp[:])

        # lt0 matmuls
        for dt in range(DT):
            for nh in range(2):
                nc.tensor.matmul(
                    score_ps[0][nh][:], qpT[:, 0, dt, :],
                    xt_sb[:, dt, nh * 512:(nh + 1) * 512],
                    start=(dt == 0), stop=(dt == DT - 1))
        softmax_front(0)     # AG-m(0) + exp(0) overlap lt1 matmuls below
        # lt1 matmuls
        for dt in range(DT):
            for nh in range(2):
                nc.tensor.matmul(
                    score_ps[1][nh][:], qpT[:, 1, dt, :],
                    xt_sb[:, dt, nh * 512:(nh + 1) * 512],
                    start=(dt == 0), stop=(dt == DT - 1))
        softmax_front(1)
        softmax_mexch()
        transpose_p(0, 0, 4)
        transpose_p(1, 0, 4)
        # a-half of pT (local n-tiles 0-3, both l-tiles) + stats ride-along
        s_pack = small.tile([128, 2 * LT], f32, name="s_pack")
        nc.vector.tensor_copy(s_pack[:, 0:LT], m_loc[:])
        nc.vector.tensor_copy(s_pack[:, LT:2 * LT], s_loc[:])
        nc.vector.tensor_copy(pT_loc[:][:, NTL * L:NTL * L + 8].bitcast(u16),
                              s_pack[:].bitcast(u16))
        nc.scalar.dma_start(agp_ia.ap()[:, 0:4 * L],
                            pT_loc[:][:, 0:4 * L].bitcast(u16))
        nc.scalar.dma_start(agp_ia.ap()[:, 4 * L:4 * L + 8],
                            pT_loc[:][:, NTL * L:NTL * L + 8].bitcast(u16))
        nc.gpsimd.collective_compute(
            "AllGather", ALU.bypass, replica_groups=RG,
            ins=[agp_ia.ap().opt()], outs=[agp_oa.ap().opt()])
        transpose_p(0, 4, 8)
        transpose_p(1, 4, 8)
        nc.scalar.dma_start(agp_ib.ap(),
                            pT_loc[:][:, 4 * L:NTL * L].bitcast(u16))
        nc.gpsimd.collective_compute(
            "AllGather", ALU.bypass, replica_groups=RG,
            ins=[agp_ib.ap().opt()], outs=[agp_ob.ap().opt()])
        scps_cm.__exit__(None, None, None)
        ph2ps_cm.__exit__(None, None, None)

        qpT_cm.__exit__(None, None, None)
        xt_cm.__exit__(None, None, None)

        # ------------- phase 3: uT[ds_j, L] = x[:, ds_j].T @ pT (full n) -----
        # pT is idx-ordered: idx = h*32 + r*4 + t  <->  global nt = 8r + 4h + t
        # (host permutes xc to match, so consumption is sequential in idx).
        pTp_cm = tc.tile_pool(name="pTp", bufs=1)
        pTp = pTp_cm.__enter__()
        pT = pTp.tile([128, NTA, L], f16, name="pT")             # full pT (4MB)
        pT5 = pT[:].bitcast(u16).rearrange("p (h r t) l -> p h r t l", h=2, r=8)
        agpo_pa = agpo_rea[:, :, 0:4 * L].rearrange("p r (t l) -> p r t l", t=4)
        for c in range(4):
            nc.gpsimd.dma_start(pT5[:, 0, c * 2:(c + 1) * 2, :, :],
                                agpo_pa[:, c * 2:(c + 1) * 2, :, :])
        ms32 = small.tile([128, 32], f32, name="ms32")
        nc.gpsimd.dma_start(
            ms32[:].bitcast(u16).rearrange("p (r c) -> p r c", r=8),
            agpo_rea[:, :, 4 * L:4 * L + 8])
        for c in range(4):
            nc.gpsimd.dma_start(pT5[:, 1, c * 2:(c + 1) * 2, :, :],
                                agpo_reb[:, c * 2:(c + 1) * 2, :, :])

        with tc.tile_pool(name="ph3xc", bufs=1) as ph3xc, \
             tc.tile_pool(name="ph3ps", bufs=1, space="PSUM") as ph3ps:
            xc_sb = ph3xc.tile([128, NTA, DS], f16)
            for c in range(4):
                nc.sync.dma_start(xc_sb[:, c * 16:(c + 1) * 16, :],
                                  xc_re[:, c * 16:(c + 1) * 16, :])
            psu = [ph3ps.tile([128, L], f32, name=f"psu{i}") for i in range(4)]
            # pass 1: d-column tiles 0-1 over all n -> uT half a early, so
            # AG-uT-a's mesh overlaps pass 2 below
            for idx in range(NTA):
                for ci in range(2):
                    nc.tensor.matmul(
                        psu[ci][:], xc_sb[:, idx, ci * 128:(ci + 1) * 128],
                        pT[:, idx, :], start=(idx == 0), stop=(idx == NTA - 1))
            for ci in range(2):
                nc.vector.tensor_copy(uT_loc[:, ci, :], psu[ci][:])
            nc.scalar.dma_start(agu_ia.ap().rearrange("p (t l) -> p t l", t=2),
                                uT_loc[:, 0:2, :])
            nc.gpsimd.collective_compute(
                "AllGather", ALU.bypass, replica_groups=RG,
                ins=[agu_ia.ap().opt()], outs=[agu_oa.ap().opt()])
            # pass 2: d-column tiles 2-3 (re-streams resident xc/pT)
            for idx in range(NTA):
                for ci in range(2, 4):
                    nc.tensor.matmul(
                        psu[ci][:], xc_sb[:, idx, ci * 128:(ci + 1) * 128],
                        pT[:, idx, :], start=(idx == 0), stop=(idx == NTA - 1))
            for ci in range(2, 4):
                nc.vector.tensor_copy(uT_loc[:, ci, :], psu[ci][:])
            nc.scalar.dma_start(agu_ib.ap().rearrange("p (t l) -> p t l", t=2),
                                uT_loc[:, 2:4, :])
            nc.gpsimd.collective_compute(
                "AllGather", ALU.bypass, replica_groups=RG,
                ins=[agu_ib.ap().opt()], outs=[agu_ob.ap().opt()])
        pTp_cm.__exit__(None, None, None)

        # S = sum_j s_j * exp(m_j - M) from the AG-pT ride-along; 1/S
        ms_all = ms32[:].rearrange("p (r c) -> p r c", r=8)
        f_all = small.tile([128, 8, LT], f32, name="f_all")
        Sg = small.tile([128, LT], f32, name="Sg")
        rec = small.tile([128, LT], f32, name="rec")
        for lt in range(LT):
            nc.scalar.activation(f_all[:, :, lt:lt + 1], ms_all[:, :, lt:lt + 1],
                                 AF.Exp, bias=negMg[:, lt:lt + 1])
            nc.vector.tensor_tensor(f_all[:, :, lt:lt + 1],
                                    f_all[:, :, lt:lt + 1],
                                    ms_all[:, :, LT + lt:LT + lt + 1], ALU.mult)
            nc.vector.tensor_reduce(Sg[:, lt:lt + 1], f_all[:, :, lt:lt + 1],
                                    axis=AX.XY, op=ALU.add)
        nc.vector.reciprocal(rec[:], Sg[:])

        # ------------- phase 4: out = (ctxT/S).T @ WvT + Wv_b ----------------
        ctx_cm = tc.tile_pool(name="ctxp", bufs=1)
        ctxp = ctx_cm.__enter__()
        # ctxT is idx-ordered: idx = h*16 + r*2 + t  <->  global dt = 4r+2h+t
        ctxT = ctxp.tile([128, DT, L], f16, name="ctxT")         # full uT (2MB)
        ctx5 = ctxT[:].rearrange("p (h r t) l -> p h r t l", h=2, r=8)
        for c in range(4):
            nc.gpsimd.dma_start(ctx5[:, 0, c * 2:(c + 1) * 2, :, :],
                                aguo_rea[:, c * 2:(c + 1) * 2, :, :])
        for c in range(4):
            nc.gpsimd.dma_start(ctx5[:, 1, c * 2:(c + 1) * 2, :, :],
                                aguo_reb[:, c * 2:(c + 1) * 2, :, :])
        with tc.tile_pool(name="ph4ps", bufs=1, space="PSUM") as ph4ps, \
             tc.tile_pool(name="ph4o", bufs=2) as ph4o:
            po = [ph4ps.tile([128, DS], f32, name=f"po{i}") for i in range(LT)]
            for h in range(2):
                for r in range(NCORES):
                    for t in range(2):
                        idx = h * 16 + r * 2 + t
                        dt = 4 * r + 2 * h + t
                        first = (h == 0 and r == 0 and t == 0)
                        last = (h == 1 and r == NCORES - 1 and t == 1)
                        for lt in range(LT):
                            nc.tensor.matmul(
                                po[lt][:],
                                ctxT[:, idx, lt * 128:(lt + 1) * 128],
                                wv_sb[:, dt, :], start=first, stop=last)
            for lt in range(LT):
                o_sb = ph4o.tile([128, DS], f32)
                nc.scalar.activation(o_sb[:], po[lt][:], AF.Copy,
                                     scale=rec[:, lt:lt + 1])
                nc.vector.tensor_tensor(o_sb[:], o_sb[:], bias_sb[:], ALU.add)
                nc.scalar.dma_start(t_out[lt * 128:(lt + 1) * 128, :], o_sb[:])
        ctx_cm.__exit__(None, None, None)
        wv_cm.__exit__(None, None, None)

    if split_waits:
        _split_waits(nc, mybir, bass_rust)
        _NC = nc
    return nc


def _tile128(a):
    """[T*128, F] -> [128, T*F] so each partition's bytes are contiguous."""
    t = a.shape[0] // 128
    return np.ascontiguousarray(
        a.reshape(t, 128, a.shape[1]).transpose(1, 0, 2).reshape(128, -1))


# phase-3 consumption order: idx = h*32 + r*4 + t <-> global nt = 8r + 4h + t
XC_PERM = [8 * r + 4 * h + t for h in range(2) for r in range(8)
           for t in range(4)]

last_results = None


def kernel(src_prompts, query, Wk_w, Wk_b, Wv_w, Wv_b):
    global last_results
    from concourse.bass_utils import run_bass_kernel_spmd

    nc = _build()

    x = np.asarray(src_prompts, dtype=np.float32)[0]
    q = np.asarray(query, dtype=np.float32)
    wk = np.asarray(Wk_w, dtype=np.float32)
    wv = np.asarray(Wv_w, dtype=np.float32)
    wvb = np.asarray(Wv_b, dtype=np.float32)
    # Wk_b shifts every score row by a constant -> cancels in softmax.

    x16 = x.astype(np.float16)
    qts = _tile128(np.ascontiguousarray(q.T).astype(np.float16))
    in_maps = []
    for j in range(NCORES):
        ns, ds = slice(j * NS, (j + 1) * NS), slice(j * DS, (j + 1) * DS)
        in_maps.append({
            "qts": qts,
            "wk": _tile128(wk[:, ds].astype(np.float16)),
            "xt": _tile128(np.ascontiguousarray(x16[ns].T)),
            "xc": _tile128(np.ascontiguousarray(
                x16[:, ds].reshape(64, 128, DS)[XC_PERM].reshape(N, DS))),
            "wvt": _tile128(np.ascontiguousarray(wv[ds].T).astype(np.float16)),
            "wvb": np.ascontiguousarray(wvb[ds][None, :]),
        })

    res = run_bass_kernel_spmd(nc, in_maps, core_ids=list(range(NCORES)))
    last_results = res
    out = np.concatenate([res.results[j]["out"] for j in range(NCORES)], axis=1)
    return out[None, :, :]
